# revision 1
# baseline (speedup 1.0000x reference)
"""DeltaModel Trainium2 kernel.

Pipeline per core (2 batch elements per core, 8 cores data-parallel):
  embed-gather (one-hot matmul) -> FFN -> LayerNorm -> chunked delta-rule
  fast-weight recurrence (C=128 chunks, WY representation, block-diag
  Jacobi solve + exact block-Horner outer correction) -> readout head.

Math per chunk (K rows = keys token-major [128,64], beta = 1/(||k||^2+eps)):
  A = strict_tril(diag(beta) K K^T)            (A[t,s], t>s)
  X = (I+A)^-1 [K | Kb],  W = X[:,:64], Z = X[:,64:]
  M_c+1^T = M_c^T + K^T W - (K^T Z) M_c^T
  ctx = M_final q ; out = (ctx Wr + br) Wo + bo
(I+A)^-1 applied via: T_bd = (I+A_bd)^-1 (m Jacobi iters, A_bd = 32-block
diagonal part), N = T_bd A_off, then (I+N)^-1 = I-N+N^2-N^3 exactly.
"""

import numpy as np

H = 64
V = 64
B = 16
L = 2048
NCORES = 8
BPC = B // NCORES          # batch per core = 2
NT = 16                    # chunks of 128 tokens per batch element
C = 128                    # chunk length
M_SOLVE = 8                # Jacobi iterations for block-diag solve
PKW = 708
LN_EPS = 1e-5
D_EPS = 1e-6

_CACHE = {}


def _build_nc(legalize=True):
    import concourse.bass as bass
    import concourse.mybir as mybir
    import concourse.tile as tile
    from concourse import masks

    dt = mybir.dt
    f32 = dt.float32
    bf16 = dt.bfloat16
    i32 = dt.int32
    Alu = mybir.AluOpType
    Act = mybir.ActivationFunctionType
    Axis = mybir.AxisListType

    nc = bass.Bass()

    seq_p = nc.declare_dram_parameter("seq", [BPC, L, 2], i32, isOutput=False)
    pk_p = nc.declare_dram_parameter("pk", [128, PKW], f32, isOutput=False)
    out_p = nc.declare_dram_parameter("out", [BPC, V], f32, isOutput=True)

    from contextlib import ExitStack
    with tile.TileContext(nc) as tc, ExitStack() as est:
        persist = est.enter_context(tc.tile_pool(name="persist", bufs=1))
        _tcount = [0]
        def _tile(shape, dtype, name=None):
            n = name or f"t{_tcount[0]}"
            _tcount[0] += 1
            return persist.tile(shape, dtype, name=n, tag=n)
        # ---------- constants ----------
        I64r = _tile([64, 64], f32)
        masks.make_identity(nc, I64r[:])
        I64 = _tile([64, 64], f32)
        nc.vector.tensor_copy(I64[:], I64r[:])
        I128r = _tile([128, 128], f32)
        masks.make_identity(nc, I128r[:])
        I128 = _tile([128, 128], f32)
        nc.vector.tensor_copy(I128[:], I128r[:])
        I128b = _tile([128, 128], bf16)
        nc.vector.tensor_copy(I128b[:], I128r[:])

        # block-diag strict-upper mask (keep S[s,t] with s<t, same 32-block)
        mask_bdsu = _tile([128, 128], f32)
        nc.gpsimd.memset(mask_bdsu[:], 0.0)
        for blk in range(4):
            sub = mask_bdsu[32 * blk:32 * blk + 32, 32 * blk:32 * blk + 32]
            # keep in_ (0) where (p - y) >= 0, else fill 1.0  -> upper strict
            nc.gpsimd.affine_select(
                out=sub, in_=sub, compare_op=Alu.is_ge, fill=1.0,
                base=0, pattern=[[-1, 32]], channel_multiplier=1)

        # off-block strict-lower mask (keep A[t,s] with s<t, different block)
        mask_offsl = _tile([128, 128], f32)
        nc.gpsimd.memset(mask_offsl[:], 1.0)
        # zero everything except strict lower (keep where (p - y) > 0)
        nc.gpsimd.affine_select(
            out=mask_offsl[:], in_=mask_offsl[:], compare_op=Alu.is_gt,
            fill=0.0, base=0, pattern=[[-1, 128]], channel_multiplier=1)
        for blk in range(4):
            nc.gpsimd.memset(
                mask_offsl[32 * blk:32 * blk + 32, 32 * blk:32 * blk + 32], 0.0)

        # row mask: 1 everywhere except partition 127 -> 0 (last key masked)
        rowmask = _tile([128, 1], f32)
        nc.gpsimd.memset(rowmask[:], 1.0)
        nc.gpsimd.affine_select(
            out=rowmask[:], in_=rowmask[:], compare_op=Alu.is_gt, fill=0.0,
            base=127, pattern=[[0, 1]], channel_multiplier=-1)

        iota_i = _tile([64, 1], i32)
        nc.gpsimd.iota(iota_i[:], pattern=[[0, 1]], base=0, channel_multiplier=1)
        iota_f = _tile([64, 1], f32)
        nc.vector.tensor_copy(iota_f[:], iota_i[:])

        ones1x64r = _tile([1, 64], f32)
        nc.gpsimd.memset(ones1x64r[:], 1.0)
        ones1x64 = _tile([1, 64], f32)
        nc.vector.tensor_copy(ones1x64[:], ones1x64r[:])
        ones1x128r = _tile([1, 128], f32)
        nc.gpsimd.memset(ones1x128r[:], 1.0)
        ones1x128 = _tile([1, 128], f32)
        nc.vector.tensor_copy(ones1x128[:], ones1x128r[:])
        one11r = _tile([1, 1], f32)
        nc.gpsimd.memset(one11r[:], 1.0)
        one11 = _tile([1, 1], f32)
        nc.vector.tensor_copy(one11[:], one11r[:])
        epsc = _tile([128, 1], f32)
        nc.gpsimd.memset(epsc[:], LN_EPS)

        # ---------- parameters via one packed DMA ----------
        pk_raw = _tile([128, PKW], f32, name="pk_raw")
        nc.sync.dma_start(pk_raw[:], pk_p[:])
        pk_sb = _tile([128, PKW], f32, name="pk_sb")
        nc.vector.tensor_copy(pk_sb[:], pk_raw[:])
        W2 = pk_sb[:, 0:64]
        W1 = pk_sb[0:64, 64:192]
        emb = pk_sb[0:64, 192:256]
        Wr = pk_sb[0:64, 256:320]
        Wo = pk_sb[0:64, 320:384]
        b1c = pk_sb[:, 384:385]
        gar = pk_sb[0:1, 385:449]
        ber = pk_sb[0:1, 449:513]
        b2r = pk_sb[0:1, 513:577]
        brr = pk_sb[0:1, 577:641]
        bor = pk_sb[0:1, 641:705]

        seqf = []
        for b in range(BPC):
            si = _tile([1, L], i32, name=f"seqi{b}")
            nc.sync.dma_start(si[:], seq_p[b:b + 1, :, 0])
            sf = _tile([1, L], f32, name=f"seqf{b}")
            nc.vector.tensor_copy(sf[:], si[:])
            seqf.append(sf)

        # psum pools
        pp = est.enter_context(tc.tile_pool(name="pp", bufs=2, space="PSUM"))

        # sbuf pools
        sb_kt = est.enter_context(tc.tile_pool(name="sb_kt", bufs=8))
        sb_sbd = est.enter_context(tc.tile_pool(name="sb_sbd", bufs=6))
        sb_x = est.enter_context(tc.tile_pool(name="sb_x", bufs=8))
        sb_v = est.enter_context(tc.tile_pool(name="sb_v", bufs=8))
        sb_fzk = est.enter_context(tc.tile_pool(name="sb_fzk", bufs=8))
        sb_mt = est.enter_context(tc.tile_pool(name="sb_mt", bufs=4))
        sb_sc = est.enter_context(tc.tile_pool(name="sb_sc", bufs=4))
        sb_small = est.enter_context(tc.tile_pool(name="sb_small", bufs=8))

        # broadcast gamma/beta to [128, 64]
        gb_ps = pp.tile([128, H], f32, name="gb_ps", tag="psmall")
        nc.tensor.matmul(gb_ps[:], lhsT=ones1x128[:], rhs=gar, start=True, stop=True)
        gamma_bc = _tile([128, H], f32)
        nc.vector.tensor_copy(gamma_bc[:], gb_ps[:])
        bb_ps = pp.tile([128, H], f32, name="bb_ps", tag="psmall")
        nc.tensor.matmul(bb_ps[:], lhsT=ones1x128[:], rhs=ber, start=True, stop=True)
        beta_bc = _tile([128, H], f32)
        nc.vector.tensor_copy(beta_bc[:], bb_ps[:])

        # ---------- embedding + FFN part 1 (feature-major) ----------
        hT = [_tile([H, L], f32, name=f"hT{b}") for b in range(BPC)]
        g1 = [_tile([2 * H, L], f32, name=f"g1{b}") for b in range(BPC)]
        oh_sb = est.enter_context(tc.tile_pool(name="oh_sb", bufs=4))

        for b in range(BPC):
            for t0 in range(0, L, C):
                bc_ps = pp.tile([V, C], f32, name="bc_ps", tag="psmall")
                nc.tensor.matmul(bc_ps[:], lhsT=ones1x64[:],
                                 rhs=seqf[b][:, t0:t0 + C],
                                 start=True, stop=True)
                oh = oh_sb.tile([V, C], f32, name="oh")
                nc.vector.tensor_scalar(
                    out=oh[:], in0=bc_ps[:], scalar1=iota_f[:], scalar2=None,
                    op0=Alu.is_equal)
                ht_ps = pp.tile([H, C], f32, name="ht_ps", tag="psmall")
                nc.tensor.matmul(ht_ps[:], lhsT=emb, rhs=oh[:],
                                 start=True, stop=True)
                nc.scalar.copy(hT[b][:, t0:t0 + C], ht_ps[:])

            for t0 in range(0, L, 512):
                g_ps = pp.tile([2 * H, 512], f32, name="g_ps", tag="pxg", bufs=1)
                nc.tensor.matmul(g_ps[:], lhsT=W1, rhs=hT[b][:, t0:t0 + 512],
                                 start=True, stop=True)
                nc.vector.tensor_scalar(
                    out=g1[b][:, t0:t0 + 512], in0=g_ps[:], scalar1=b1c,
                    scalar2=0.0, op0=Alu.add, op1=Alu.max)

        # ---------- per 128-token tile: x, LN, keys, chunk math ----------
        R = [_tile([128, 256], bf16, name=f"R{i}") for i in range(BPC * NT)]
        q_row = [_tile([1, H], f32, name=f"q{b}") for b in range(BPC)]
        q_raw = [_tile([1, H], bf16, name=f"qr{b}") for b in range(BPC)]
        mt_cur = [None] * BPC

        for i in range(BPC * NT):
            b, c = i // NT, i % NT
            t0 = c * C
            # x = h + relu(h W1 + b1) W2 + b2   (token-major via PE)
            x_ps = pp.tile([128, H], f32, name="x_ps", tag="pxg2", bufs=1)
            nc.tensor.matmul(x_ps[:], lhsT=g1[b][:, t0:t0 + C], rhs=W2,
                             start=True, stop=False)
            nc.tensor.matmul(x_ps[:], lhsT=hT[b][:, t0:t0 + C], rhs=I64[:],
                             start=False, stop=False)
            nc.tensor.matmul(x_ps[:], lhsT=ones1x128[:], rhs=b2r,
                             start=False, stop=True)
            # LayerNorm
            s1 = sb_small.tile([128, 1], f32, name="s1")
            nc.vector.tensor_reduce(s1[:], x_ps[:], axis=Axis.X, op=Alu.add)
            mu = sb_small.tile([128, 1], f32, name="mu")
            nc.vector.tensor_scalar_mul(mu[:], s1[:], 1.0 / H)
            xc = sb_sc.tile([128, H], f32, name="xc")
            nc.vector.tensor_scalar(out=xc[:], in0=x_ps[:], scalar1=mu[:],
                                    scalar2=None, op0=Alu.subtract)
            sqs = sb_sc.tile([128, H], f32, name="sqs")
            ssq = sb_small.tile([128, 1], f32, name="ssq")
            nc.scalar.activation(sqs[:], xc[:], Act.Square, accum_out=ssq[:])
            sroot = sb_small.tile([128, 1], f32, name="sroot")
            nc.scalar.activation(sroot[:], ssq[:], Act.Sqrt,
                                 bias=epsc[:], scale=1.0 / H)
            rstd = sb_small.tile([128, 1], f32, name="rstd")
            nc.vector.reciprocal(rstd[:], sroot[:])
            kk = sb_sc.tile([128, H], f32, name="kk")
            nc.gpsimd.tensor_scalar(out=kk[:], in0=xc[:], scalar1=rstd[:],
                                    scalar2=None, op0=Alu.mult)
            kg = sb_sc.tile([128, H], f32, name="kg")
            nc.vector.tensor_mul(kg[:], kk[:], gamma_bc[:])
            nc.vector.tensor_add(R[i][:, 0:H], kg[:], beta_bc[:])
            if c == NT - 1:
                # query = last token's normalized h; then mask it out of keys
                nc.sync.dma_start(q_raw[b][:], R[i][127:128, 0:H])
                nc.vector.tensor_copy(q_row[b][:], q_raw[b][:])
                nc.vector.tensor_scalar(
                    out=R[i][:, 0:H], in0=R[i][:, 0:H], scalar1=rowmask[:],
                    scalar2=None, op0=Alu.mult)
            # beta_t and Kb
            ssk = sb_small.tile([128, 1], f32, name="ssk")
            sqk = sb_sc.tile([128, H], f32, name="sqk")
            nc.scalar.activation(sqk[:], R[i][:, 0:H], Act.Square,
                                 accum_out=ssk[:])
            btv = sb_small.tile([128, 1], f32, name="btv")
            nc.vector.tensor_scalar_add(btv[:], ssk[:], D_EPS)
            beta_t = sb_small.tile([128, 1], f32, name="beta_t")
            nc.vector.reciprocal(beta_t[:], btv[:])
            nc.vector.tensor_scalar(out=R[i][:, H:2 * H], in0=R[i][:, 0:H],
                                    scalar1=beta_t[:], scalar2=None,
                                    op0=Alu.mult)

            # ---- transposes ----
            kt_ps = pp.tile([H, 128], f32, name="kt_ps", tag="psmall")
            nc.tensor.matmul(kt_ps[:], lhsT=R[i][:, 0:H], rhs=I128b[:],
                             start=True, stop=True)
            KT = sb_kt.tile([H, 128], bf16, name="KT")
            nc.scalar.copy(KT[:], kt_ps[:])
            kbt_ps = pp.tile([H, 128], f32, name="kbt_ps", tag="psmall")
            nc.tensor.matmul(kbt_ps[:], lhsT=R[i][:, H:2 * H], rhs=I128b[:],
                             start=True, stop=True)
            KbT = sb_kt.tile([H, 128], bf16, name="KbT")
            nc.scalar.copy(KbT[:], kbt_ps[:])

            # ---- S (stationary orientation) and A_off ----
            s_ps = pp.tile([128, 128], f32, name="s_ps", tag="psa", bufs=1)
            nc.tensor.matmul(s_ps[:], lhsT=KT[:], rhs=KbT[:],
                             start=True, stop=True)
            S_bd = sb_sbd.tile([128, 128], bf16, name="S_bd")
            nc.vector.tensor_mul(S_bd[:], s_ps[:], mask_bdsu[:])
            a_ps = pp.tile([128, 128], f32, name="a_ps", tag="psa", bufs=1)
            nc.tensor.matmul(a_ps[:], lhsT=KbT[:], rhs=KT[:],
                             start=True, stop=True)
            nc.vector.tensor_mul(R[i][:, 2 * H:4 * H], a_ps[:], mask_offsl[:])

            # ---- block-diag Jacobi solve: X = R - A_bd X ----
            prev = R[i][:]
            for j in range(M_SOLVE):
                sol_ps = pp.tile([128, 256], f32, name="sol_ps", tag="psol", bufs=3)
                nc.tensor.matmul(sol_ps[:], lhsT=S_bd[:], rhs=prev,
                                 start=True, stop=True)
                X = sb_x.tile([128, 256], bf16, name="X")
                nc.vector.tensor_sub(X[:], R[i][:], sol_ps[:])
                prev = X[:]

            # ---- NT = N^T via PE transpose ----
            nt_ps = pp.tile([128, 128], f32, name="nt_ps", tag="psa", bufs=1)
            nc.tensor.matmul(nt_ps[:], lhsT=prev[:, 2 * H:4 * H], rhs=I128b[:],
                             start=True, stop=True)
            NTt = sb_sbd.tile([128, 128], bf16, name="NTt")
            nc.scalar.copy(NTt[:], nt_ps[:])

            # ---- outer Horner: V = Y - N V  (3x, exact) ----
            Y = prev[:, 0:2 * H]
            prevV = Y
            for j in range(3):
                o_ps = pp.tile([128, 2 * H], f32, name="o_ps", tag="psol", bufs=3)
                nc.tensor.matmul(o_ps[:], lhsT=NTt[:], rhs=prevV,
                                 start=True, stop=True)
                Vt = sb_v.tile([128, 2 * H], bf16, name="Vt")
                nc.vector.tensor_sub(Vt[:], Y, o_ps[:])
                prevV = Vt[:]

            # ---- F = K^T W, ZK = Z^T K ----
            f_ps = pp.tile([H, H], f32, name="f_ps", tag="psmall")
            nc.tensor.matmul(f_ps[:], lhsT=R[i][:, 0:H], rhs=prevV[:, 0:H],
                             start=True, stop=True)
            Ft = sb_fzk.tile([H, H], f32, name="Ft")
            nc.scalar.copy(Ft[:], f_ps[:])
            zk_ps = pp.tile([H, H], f32, name="zk_ps", tag="psmall")
            nc.tensor.matmul(zk_ps[:], lhsT=prevV[:, H:2 * H], rhs=R[i][:, 0:H],
                             start=True, stop=True)
            ZKt = sb_fzk.tile([H, H], f32, name="ZKt")
            nc.scalar.copy(ZKt[:], zk_ps[:])

            # ---- sequential state update ----
            if c == 0:
                mt_cur[b] = Ft
            else:
                sq_ps = pp.tile([H, H], f32, name="sq_ps", tag="psmall")
                nc.tensor.matmul(sq_ps[:], lhsT=ZKt[:], rhs=mt_cur[b][:],
                                 start=True, stop=True)
                tmp = sb_mt.tile([H, H], f32, name="tmp_mt")
                nc.vector.tensor_sub(tmp[:], Ft[:], sq_ps[:])
                mt_new = sb_mt.tile([H, H], f32, name="mt_new")
                nc.vector.tensor_add(mt_new[:], mt_cur[b][:], tmp[:])
                mt_cur[b] = mt_new

        # ---------- readout head ----------
        for b in range(BPC):
            qt_ps = pp.tile([H, 1], f32, name="qt_ps", tag="psmall")
            nc.tensor.matmul(qt_ps[:], lhsT=q_row[b][:], rhs=one11[:],
                             start=True, stop=True)
            qT = sb_small.tile([H, 1], f32, name="qT")
            nc.vector.tensor_copy(qT[:], qt_ps[:])
            cx_ps = pp.tile([H, 1], f32, name="cx_ps", tag="psmall")
            nc.tensor.matmul(cx_ps[:], lhsT=mt_cur[b][:], rhs=qT[:],
                             start=True, stop=True)
            ctx = sb_small.tile([H, 1], f32, name="ctx")
            nc.vector.tensor_copy(ctx[:], cx_ps[:])
            z_ps = pp.tile([H, 1], f32, name="z_ps", tag="psmall")
            nc.tensor.matmul(z_ps[:], lhsT=Wr, rhs=ctx[:],
                             start=True, stop=False)
            nc.tensor.matmul(z_ps[:], lhsT=brr, rhs=one11[:],
                             start=False, stop=True)
            zt = sb_small.tile([H, 1], f32, name="zt")
            nc.vector.tensor_copy(zt[:], z_ps[:])
            y_ps = pp.tile([V, 1], f32, name="y_ps", tag="psmall")
            nc.tensor.matmul(y_ps[:], lhsT=Wo, rhs=zt[:],
                             start=True, stop=False)
            nc.tensor.matmul(y_ps[:], lhsT=bor, rhs=one11[:],
                             start=False, stop=True)
            yt = sb_small.tile([V, 1], f32, name="yt")
            nc.vector.tensor_copy(yt[:], y_ps[:])
            nc.sync.dma_start(out_p[b, :, None], yt[:])

    if legalize:
        _legalize_waits(nc, mybir)
    return nc


def _legalize_waits(nc, mybir):
    """This walrus build encodes at most one sync-wait per instruction.
    Split multi-wait instructions into single-wait NoOp prefixes on the
    same engine (engine queues execute in order, so semantics hold)."""
    k = 0
    for blk in nc.main_func.blocks:
        insts = blk.instructions
        out = []
        changed = False
        for inst in list(insts):
            si = inst.sync_info
            waits = list(si.on_wait) if si is not None and si.on_wait else []
            if len(waits) > 1:
                for w in waits[:-1]:
                    nop = mybir.InstNoOp(name=f"I-wsplit-{k}", ins=[], outs=[])
                    k += 1
                    nop.engine = inst.engine
                    nop.sync_info = mybir.SyncInfo(on_wait=[w], on_update=[])
                    out.append(nop)
                si.on_wait = [waits[-1]]
                changed = True
            out.append(inst)
        if changed:
            while len(insts):
                insts.pop()
            for x in out:
                insts.append(x)


def pack_params(inputs):
    g = lambda k: np.asarray(inputs[k], dtype=np.float32)
    pk = np.zeros((128, PKW), np.float32)
    pk[:, 0:64] = g("W2")
    pk[0:64, 64:192] = g("W1")
    pk[0:64, 192:256] = g("embed")
    pk[0:64, 256:320] = g("Wr")
    pk[0:64, 320:384] = g("Wo")
    pk[:, 384] = g("b1")
    pk[0, 385:449] = g("gamma")
    pk[0, 449:513] = g("beta")
    pk[0, 513:577] = g("b2")
    pk[0, 577:641] = g("br")
    pk[0, 641:705] = g("bo")
    return np.ascontiguousarray(pk)


def _get_nc():
    if "nc" not in _CACHE:
        _CACHE["nc"] = _build_nc()
    return _CACHE["nc"]


def kernel(**inputs):
    from concourse.bass_utils import run_bass_kernel_spmd

    nc = _get_nc()
    seq = np.ascontiguousarray(np.asarray(inputs["seq"], dtype=np.int64))
    seq32 = seq.view(np.int32).reshape(B, L, 2)
    pk = pack_params(inputs)
    in_maps = []
    for core in range(NCORES):
        m = {"seq": np.ascontiguousarray(seq32[core * BPC:(core + 1) * BPC]),
             "pk": pk}
        in_maps.append(m)
    res = run_bass_kernel_spmd(nc, in_maps, core_ids=list(range(NCORES)))
    out = np.concatenate([r["out"] for r in res.results], axis=0)
    return out.astype(np.float32)


if __name__ == "__main__":
    d = np.load("/root/problem/inputs.npz")
    y = kernel(**{k: d[k] for k in d.files})
    o = np.load("/root/problem/oracle.npz")
    rel = np.abs(y - o["y"]).max() / np.abs(o["y"]).max()
    print("Relative error:", rel)



# revision 2
# speedup vs baseline: 6.1375x; 6.1375x over previous
"""DeltaModel Trainium2 kernel, v2.

Host folds the vocab-64 token table (embed->FFN->LayerNorm), gathers per-token
K / beta*K rows, and ships them in both token-major and feature-major layouts.
Device runs, per 128-token chunk (2 batch elems/core paired side by side):
  A = Kb K^T, S = A^T (swapped-operand matmuls)
  block-diag strict-lower L = A_bd; solve (I+L)^-1 via exact degree-5
  Neumann product (I + L^2 + L^4)(I - L) -> X = [W | -Z | N-part]
  outer correction (I+N)^-1 = I - N + N^2 - N^3 (exact, N^4 = 0)
  Mt <- Mt + K^T W - (K^T Z) Mt   (PSUM-accumulated, chained over chunks)
  readout y = (Mt^T q) (Wr Wo) + (br Wo + bo) in fp32.
Emission is stage-major across chunk pairs (4 groups of 4) so independent
chunks hide cross-engine semaphore latency; PSUM->SBUF traffic is balanced
across DVE / Act / Pool.
"""

import numpy as np

H = 64
V = 64
B = 16
L = 2048
NCORES = 8
BPC = 2                 # batch per core
NT = 16                 # chunks per batch elem
C = 128                 # chunk length
NBLK = 32               # jacobi block
RW = 224                # R width: K(64) | -Kb(64) | A_off(96)
GP = 4                  # pairs per group
LN_EPS = 1e-5
D_EPS = 1e-6

_CACHE = {}


def _build_nc(legalize=True):
    import concourse.bass as bass
    import concourse.mybir as mybir
    import concourse.tile as tile

    dt = mybir.dt
    f32 = dt.float32
    bf16 = dt.bfloat16
    Act = mybir.ActivationFunctionType

    nc = bass.Bass()

    kkbn_p = nc.declare_dram_parameter("kkbn", [128, NT * BPC * 128], bf16, isOutput=False)
    kt_p = nc.declare_dram_parameter("kt", [64, 2 * L], bf16, isOutput=False)
    kbt_p = nc.declare_dram_parameter("kbt", [64, 2 * L], bf16, isOutput=False)
    consts_p = nc.declare_dram_parameter("consts", [128, 832], bf16, isOutput=False)
    fpack_p = nc.declare_dram_parameter("fpack", [128, 131], f32, isOutput=False)
    out_p = nc.declare_dram_parameter("out", [BPC, V], f32, isOutput=True)

    from contextlib import ExitStack
    with tile.TileContext(nc) as tc, ExitStack() as est:
        persist = est.enter_context(tc.tile_pool(name="persist", bufs=1))

        R_all = persist.tile([128, NT * BPC, RW], bf16, name="R_all")
        KT = persist.tile([64, 2 * L], bf16, name="KT")
        KbT = persist.tile([64, 2 * L], bf16, name="KbT")
        consts = persist.tile([128, 832], bf16, name="consts")
        fpack = persist.tile([128, 131], f32, name="fpack")
        Mt_f32 = persist.tile([64, 2 * H], f32, name="Mt_f32")
        ctx_sb = persist.tile([64, 2], f32, name="ctx_sb")
        y_sb = persist.tile([1, 2 * V], f32, name="y_sb")

        I128 = consts[:, 0:128]
        I64 = consts[0:64, 0:64]
        M_BDL = consts[:, 128:384]     # [128,256] pair mask, -1 strict-lower blkdiag
        M_BDU = consts[:, 384:640]     # -1 strict-upper blkdiag
        M_OFF = consts[:, 640:832]     # [128,192] pair mask, +1 off-block cols 0:96
        WrWo = fpack[0:64, 0:64]
        ONES11 = fpack[0:1, 66:67]
        BIAS2 = fpack[0:1, 67:131]

        # ---------------- DMAs (group-pipelined) ----------------
        nc.sync.dma_start(consts[:], consts_p[:])
        for g in range(NT // GP):
            c0 = g * GP
            tok = c0 * 128
            for b in range(2):
                o = b * L + tok
                nc.sync.dma_start(KT[:, o:o + GP * 128], kt_p[:, o:o + GP * 128])
                nc.sync.dma_start(KbT[:, o:o + GP * 128], kbt_p[:, o:o + GP * 128])
            nc.sync.dma_start(
                R_all[:, 2 * c0:2 * (c0 + GP), 0:128],
                kkbn_p[:, c0 * 256:(c0 + GP) * 256])
        nc.sync.dma_start(fpack[:], fpack_p[:])

        # ---------------- pools ----------------
        pp = est.enter_context(tc.tile_pool(name="pp", bufs=6, space="PSUM"))
        sb_sbd = est.enter_context(tc.tile_pool(name="sb_sbd", bufs=6))
        sb_abd = est.enter_context(tc.tile_pool(name="sb_abd", bufs=6))
        sb_s2 = est.enter_context(tc.tile_pool(name="sb_s2", bufs=6))
        sb_x1 = est.enter_context(tc.tile_pool(name="sb_x1", bufs=6))
        sb_x2 = est.enter_context(tc.tile_pool(name="sb_x2", bufs=6))
        sb_x3 = est.enter_context(tc.tile_pool(name="sb_x3", bufs=6))
        sb_nt = est.enter_context(tc.tile_pool(name="sb_nt", bufs=6))
        sb_v1 = est.enter_context(tc.tile_pool(name="sb_v1", bufs=4))
        sb_v2 = est.enter_context(tc.tile_pool(name="sb_v2", bufs=4))
        sb_v3 = est.enter_context(tc.tile_pool(name="sb_v3", bufs=6))
        sb_zk = est.enter_context(tc.tile_pool(name="sb_zk", bufs=4))
        sb_mt = est.enter_context(tc.tile_pool(name="sb_mt", bufs=4))

        st = [dict() for _ in range(NT)]   # per-pair tile state
        mt_prev = [None]                   # rolling Mt tile

        def ring(shape, name):
            return pp.tile(shape, f32, name=name, tag="ring", bufs=6)

        # ---------------- stages ----------------
        def sA(c):
            # per-chunk engine routes: alternate DVE (False) / Act (True) by
            # parity so every stage feeds both elementwise engines
            ev = (c % 2 == 0)
            st[c]["rt"] = {"r1": ev, "r2": not ev, "r3": ev,
                           "h1": ev, "h2": not ev, "h3": not ev,
                           "zk": ev, "mt": not ev}
            ps_a = ring([128, 2, 128], "ps_a")
            ps_s = ring([128, 2, 128], "ps_s")
            for b in range(2):
                kts = KT[:, b * L + c * 128:b * L + (c + 1) * 128]
                kbts = KbT[:, b * L + c * 128:b * L + (c + 1) * 128]
                nc.tensor.matmul(ps_a[:, b, :], lhsT=kbts, rhs=kts, start=True, stop=True)
                nc.tensor.matmul(ps_s[:, b, :], lhsT=kts, rhs=kbts, start=True, stop=True)
            st[c]["ps_a"] = ps_a
            st[c]["ps_s"] = ps_s

        def sMask(c):
            ps_a, ps_s = st[c]["ps_a"], st[c]["ps_s"]
            sbd = sb_sbd.tile([128, 256], bf16, name="sbd")
            nc.vector.tensor_mul(sbd[:], ps_s[:, :, :], M_BDU)
            abd = sb_abd.tile([128, 256], bf16, name="abd")
            nc.vector.tensor_mul(abd[:], ps_a[:, :, :], M_BDL)
            nc.vector.tensor_mul(
                R_all[:, 2 * c:2 * c + 2, 128:224], ps_a[:, :, 0:96], M_OFF)
            st[c]["sbd"] = sbd
            st[c]["abd"] = abd

        def sS2(c):
            sbd, abd = st[c]["sbd"], st[c]["abd"]
            ps = ring([128, 2, 128], "ps_s2")
            for b in range(2):
                nc.tensor.matmul(ps[:, b, :], lhsT=abd[:, 128 * b:128 * (b + 1)],
                                 rhs=sbd[:, 128 * b:128 * (b + 1)],
                                 start=True, stop=True)
            s2 = sb_s2.tile([128, 256], bf16, name="s2")
            nc.scalar.copy(s2[:], ps[:, :, :])
            st[c]["s2"] = s2

        def _round(c, name, pool, lhs_of_b, add_in, base_in):
            """One solve round: out = base_in + L-ish @ add_in.
            Route D: bare mm + DVE tensor_add; route A: identity-fold + Act copy."""
            ps = ring([128, 2, RW], "ps_" + name)
            on_act = st[c]["rt"][name]
            for b in range(2):
                if on_act:
                    nc.tensor.matmul(ps[:, b, :], lhsT=I128, rhs=base_in(b),
                                     start=True, stop=False)
                nc.tensor.matmul(ps[:, b, :], lhsT=lhs_of_b(b), rhs=add_in(b),
                                 start=not on_act, stop=True)
            out = pool.tile([128, 2, RW], bf16, name=name)
            if on_act:
                nc.scalar.copy(out[:, :, :], ps[:, :, :])
            else:
                nc.vector.tensor_add(out[:, :, :], st[c]["base_ap"](), ps[:, :, :])
            return out

        def sR1(c):
            sbd = st[c]["sbd"]
            st[c]["base_ap"] = lambda: R_all[:, 2 * c:2 * c + 2, :]
            x1 = _round(c, "r1", sb_x1,
                        lambda b: sbd[:, 128 * b:128 * (b + 1)],
                        lambda b: R_all[:, 2 * c + b, :],
                        lambda b: R_all[:, 2 * c + b, :])
            st[c]["x1"] = x1

        def sR2(c):
            s2, x1 = st[c]["s2"], st[c]["x1"]
            st[c]["base_ap"] = lambda: x1[:, :, :]
            x2 = _round(c, "r2", sb_x2,
                        lambda b: s2[:, 128 * b:128 * (b + 1)],
                        lambda b: x1[:, b, :],
                        lambda b: x1[:, b, :])
            st[c]["x2"] = x2

        def sR3(c):
            s2, x1, x2 = st[c]["s2"], st[c]["x1"], st[c]["x2"]
            st[c]["base_ap"] = lambda: x1[:, :, :]
            x3 = _round(c, "r3", sb_x3,
                        lambda b: s2[:, 128 * b:128 * (b + 1)],
                        lambda b: x2[:, b, :],
                        lambda b: x1[:, b, :])
            st[c]["x3"] = x3

        def sNT(c):
            x3 = st[c]["x3"]
            ps = ring([96, 2, 128], "ps_nt")
            for b in range(2):
                nc.tensor.matmul(ps[:, b, :], lhsT=x3[:, b, 128:224], rhs=I128,
                                 start=True, stop=True)
            ntn = sb_nt.tile([96, 2, 128], bf16, name="ntn")
            nc.scalar.activation(ntn[:, :, :], ps[:, :, :], Act.Copy, scale=-1.0)
            st[c]["ntn"] = ntn

        def _horner(c, name, pool, vin_of_b):
            x3, ntn = st[c]["x3"], st[c]["ntn"]
            ps = ring([128, 2, 128], "ps_" + name)
            on_act = st[c]["rt"][name]
            for b in range(2):
                if on_act:
                    nc.tensor.matmul(ps[:, b, :], lhsT=I128, rhs=x3[:, b, 0:128],
                                     start=True, stop=False)
                nc.tensor.matmul(ps[:, b, :], lhsT=ntn[:, b, :],
                                 rhs=vin_of_b(b), start=not on_act, stop=True)
            out = pool.tile([128, 2, 128], bf16, name=name)
            if on_act:
                nc.scalar.copy(out[:, :, :], ps[:, :, :])
            else:
                nc.vector.tensor_add(out[:, :, :], x3[:, :, 0:128], ps[:, :, :])
            return out

        def sH1(c):
            x3 = st[c]["x3"]
            st[c]["v1"] = _horner(c, "h1", sb_v1, lambda b: x3[0:96, b, 0:128])

        def sH2(c):
            v1 = st[c]["v1"]
            st[c]["v2"] = _horner(c, "h2", sb_v2, lambda b: v1[0:96, b, :])

        def sH3(c):
            v2 = st[c]["v2"]
            st[c]["v3"] = _horner(c, "h3", sb_v3, lambda b: v2[0:96, b, :])

        def sZK(c):
            if c == 0:
                return
            v3 = st[c]["v3"]
            ps = ring([64, 2, 64], "ps_zk")
            for b in range(2):
                nc.tensor.matmul(ps[:, b, :], lhsT=v3[:, b, 64:128],
                                 rhs=R_all[:, 2 * c + b, 0:64], start=True, stop=True)
            zkn = sb_zk.tile([64, 2, 64], bf16, name="zkn")
            if st[c]["rt"]["zk"]:
                nc.scalar.copy(zkn[:, :, :], ps[:, :, :])
            else:
                nc.vector.tensor_copy(zkn[:, :, :], ps[:, :, :])
            st[c]["zkn"] = zkn

        def s12(c):
            v3 = st[c]["v3"]
            ps = pp.tile([64, 2, 64], f32, name="ps_upd", tag="upd", bufs=2)
            for b in range(2):
                nc.tensor.matmul(ps[:, b, :], lhsT=R_all[:, 2 * c + b, 0:64],
                                 rhs=v3[:, b, 0:64], start=True, stop=(c == 0))
                if c > 0:
                    zkn = st[c]["zkn"]
                    nc.tensor.matmul(ps[:, b, :], lhsT=zkn[:, b, :],
                                     rhs=mt_prev[0][:, b, :], start=False, stop=False)
                    nc.tensor.matmul(ps[:, b, :], lhsT=I64,
                                     rhs=mt_prev[0][:, b, :], start=False, stop=True)
            if c == NT - 1:
                nc.vector.tensor_copy(Mt_f32[:, :], ps[:, :, :])
            else:
                mt = sb_mt.tile([64, 2, 64], bf16, name="mt")
                if st[c]["rt"]["mt"]:
                    nc.scalar.copy(mt[:, :, :], ps[:, :, :])
                else:
                    nc.vector.tensor_copy(mt[:, :, :], ps[:, :, :])
                mt_prev[0] = mt
            st[c].clear()

        stages = [sA, sMask, sS2, sR1, sR2, sR3, sNT, sH1, sH2, sH3, sZK]
        import os
        _nstg = int(os.environ.get("KN_STAGES", "99"))
        stages = stages[:_nstg]
        _do_s12 = _nstg >= 12
        pend_slots = {1: 0, 3: 1, 5: 2, 7: 3}
        for g in range(NT // GP):
            for si, stage in enumerate(stages):
                if _do_s12 and g > 0 and si in pend_slots:
                    s12(GP * (g - 1) + pend_slots[si])
                for c in range(GP * g, GP * (g + 1)):
                    stage(c)
        if _do_s12:
            for c in range(NT - GP, NT):
                s12(c)
        else:
            nc.gpsimd.memset(Mt_f32[:, :], 0.0)

        # ---------------- readout ----------------
        ps_ctx = pp.tile([64, 2], f32, name="ps_ctx", tag="ring", bufs=6)
        for b in range(2):
            nc.tensor.matmul(ps_ctx[:, b:b + 1], lhsT=Mt_f32[:, 64 * b:64 * (b + 1)],
                             rhs=fpack[0:64, 64 + b:65 + b], start=True, stop=True)
        nc.vector.tensor_copy(ctx_sb[:], ps_ctx[:])
        ps_y = pp.tile([1, 2 * V], f32, name="ps_y", tag="ring", bufs=6)
        for b in range(2):
            nc.tensor.matmul(ps_y[0:1, 64 * b:64 * (b + 1)], lhsT=ctx_sb[:, b:b + 1],
                             rhs=WrWo, start=True, stop=False)
            nc.tensor.matmul(ps_y[0:1, 64 * b:64 * (b + 1)], lhsT=ONES11,
                             rhs=BIAS2, start=False, stop=True)
        nc.vector.tensor_copy(y_sb[:], ps_y[:])
        for b in range(2):
            nc.sync.dma_start(out_p[b:b + 1, :], y_sb[0:1, 64 * b:64 * (b + 1)])

    if legalize:
        _legalize_waits(nc, mybir)
    return nc


def _legalize_waits(nc, mybir):
    """This walrus build encodes at most one sync-wait per instruction.
    Split multi-wait instructions into single-wait NoOp prefixes on the
    same engine (engine queues execute in order, so semantics hold)."""
    k = 0
    for blk in nc.main_func.blocks:
        insts = blk.instructions
        out = []
        changed = False
        for inst in list(insts):
            si = inst.sync_info
            waits = list(si.on_wait) if si is not None and si.on_wait else []
            if len(waits) > 1:
                for w in waits[:-1]:
                    nop = mybir.InstNoOp(name=f"I-wsplit-{k}", ins=[], outs=[])
                    k += 1
                    nop.engine = inst.engine
                    nop.sync_info = mybir.SyncInfo(on_wait=[w], on_update=[])
                    out.append(nop)
                si.on_wait = [waits[-1]]
                changed = True
            out.append(inst)
        if changed:
            while len(insts):
                insts.pop()
            for x in out:
                insts.append(x)


def host_prep(inputs):
    """Fold the vocab table and gather per-token rows; returns per-core maps."""
    import ml_dtypes
    bf = ml_dtypes.bfloat16
    f64 = np.float64
    g = lambda k: np.asarray(inputs[k], f64)
    embed, W1, b1, W2, b2 = g("embed"), g("W1"), g("b1"), g("W2"), g("b2")
    gamma, beta, Wr, br, Wo, bo = (g("gamma"), g("beta"), g("Wr"), g("br"),
                                   g("Wo"), g("bo"))
    seq = np.asarray(inputs["seq"], np.int64)

    ff = np.maximum(embed @ W1 + b1, 0) @ W2 + b2
    x = embed + ff
    mu = x.mean(-1, keepdims=True)
    var = x.var(-1, keepdims=True)
    xln = ((x - mu) / np.sqrt(var + LN_EPS) * gamma + beta).astype(np.float32)
    betav = (1.0 / ((xln.astype(f64) ** 2).sum(-1) + D_EPS)).astype(np.float32)
    xlnb = xln * betav[:, None]
    WrWo = (Wr @ Wo).astype(np.float32)
    bias2 = (br @ Wo + bo).astype(np.float32)

    # constants pack
    t = np.arange(C)
    blk = t // NBLK
    m_bdl = -((t[:, None] > t[None, :]) & (blk[:, None] == blk[None, :])).astype(np.float32)
    m_bdu = m_bdl.T.copy()
    m_off = ((t[:, None] > t[None, :]) & (blk[:, None] != blk[None, :])).astype(np.float32)[:, :96]
    consts = np.zeros((128, 832), np.float32)
    consts[:, 0:128] = np.eye(128)
    consts[:, 128:256] = m_bdl
    consts[:, 256:384] = m_bdl
    consts[:, 384:512] = m_bdu
    consts[:, 512:640] = m_bdu
    consts[:, 640:736] = m_off
    consts[:, 736:832] = m_off
    consts_bf = consts.astype(bf)

    in_maps = []
    for core in range(NCORES):
        sq = seq[core * BPC:(core + 1) * BPC]        # [2, 2048]
        K = xln[sq].astype(bf).astype(np.float32)    # [2, L, 64] bf16-rounded
        Kb = xlnb[sq].astype(bf).astype(np.float32)
        q = xln[sq[:, L - 1]].astype(np.float32)     # [2, 64] before zeroing
        K[:, L - 1, :] = 0.0
        Kb[:, L - 1, :] = 0.0

        kkbn = np.empty((2, NT, C, 128), np.float32)
        kkbn[:, :, :, 0:64] = K.reshape(2, NT, C, 64)
        kkbn[:, :, :, 64:128] = -Kb.reshape(2, NT, C, 64)
        # device layout [128(t), c, b, col]
        kkbn = np.ascontiguousarray(kkbn.transpose(2, 1, 0, 3)).reshape(128, NT * BPC * 128)

        kt = np.concatenate([K[0].T, K[1].T], axis=1)     # [64, 2L]
        kbt = np.concatenate([Kb[0].T, Kb[1].T], axis=1)

        fpack = np.zeros((128, 131), np.float32)
        fpack[0:64, 0:64] = WrWo
        fpack[0:64, 64] = q[0]
        fpack[0:64, 65] = q[1]
        fpack[0, 66] = 1.0
        fpack[0, 67:131] = bias2
        in_maps.append({
            "kkbn": kkbn.astype(bf),
            "kt": kt.astype(bf),
            "kbt": kbt.astype(bf),
            "consts": consts_bf,
            "fpack": fpack,
        })
    return in_maps


def _get_nc():
    if "nc" not in _CACHE:
        _CACHE["nc"] = _build_nc()
    return _CACHE["nc"]


def kernel(**inputs):
    from concourse.bass_utils import run_bass_kernel_spmd

    nc = _get_nc()
    in_maps = host_prep(inputs)
    res = run_bass_kernel_spmd(nc, in_maps, core_ids=list(range(NCORES)))
    out = np.concatenate([r["out"] for r in res.results], axis=0)
    return out.astype(np.float32)


if __name__ == "__main__":
    d = np.load("/root/problem/inputs.npz")
    y = kernel(**{k: d[k] for k in d.files})
    o = np.load("/root/problem/oracle.npz")
    rel = np.abs(y - o["y"]).max() / np.abs(o["y"]).max()
    print("Relative error:", rel)


# revision 3
# speedup vs baseline: 7.1361x; 1.1627x over previous
"""DeltaModel Trainium2 kernel, v2.

Host folds the vocab-64 token table (embed->FFN->LayerNorm), gathers per-token
K / beta*K rows, and ships them in both token-major and feature-major layouts.
Device runs, per 128-token chunk (2 batch elems/core paired side by side):
  A = Kb K^T, S = A^T (swapped-operand matmuls)
  block-diag strict-lower L = A_bd; solve (I+L)^-1 via exact degree-5
  Neumann product (I + L^2 + L^4)(I - L) -> X = [W | -Z | N-part]
  outer correction (I+N)^-1 = I - N + N^2 - N^3 (exact, N^4 = 0)
  Mt <- Mt + K^T W - (K^T Z) Mt   (PSUM-accumulated, chained over chunks)
  readout y = (Mt^T q) (Wr Wo) + (br Wo + bo) in fp32.
Emission is stage-major across chunk pairs (4 groups of 4) so independent
chunks hide cross-engine semaphore latency; PSUM->SBUF traffic is balanced
across DVE / Act / Pool.
"""

import numpy as np

H = 64
V = 64
B = 16
L = 2048
NCORES = 8
BPC = 2                 # batch per core
NT = 16                 # chunks per batch elem
C = 128                 # chunk length
NBLK = 32               # jacobi block
RW = 224                # R width: K(64) | -Kb(64) | A_off(96)
GP = 4                  # pairs per group
LN_EPS = 1e-5
D_EPS = 1e-6

_CACHE = {}


def _build_nc(legalize=True):
    import concourse.bass as bass
    import concourse.mybir as mybir
    import concourse.tile as tile

    dt = mybir.dt
    f32 = dt.float32
    bf16 = dt.bfloat16
    Act = mybir.ActivationFunctionType

    nc = bass.Bass()

    kkbn_p = nc.declare_dram_parameter("kkbn", [128, NT * BPC * 128], bf16, isOutput=False)
    kt_p = nc.declare_dram_parameter("kt", [64, 2 * L], bf16, isOutput=False)
    kbt_p = nc.declare_dram_parameter("kbt", [64, 2 * L], bf16, isOutput=False)
    consts_p = nc.declare_dram_parameter("consts", [128, 1536], bf16, isOutput=False)
    fpack_p = nc.declare_dram_parameter("fpack", [128, 131], f32, isOutput=False)
    out_p = nc.declare_dram_parameter("out", [BPC, V], f32, isOutput=True)

    from contextlib import ExitStack
    with tile.TileContext(nc) as tc, ExitStack() as est:
        persist = est.enter_context(tc.tile_pool(name="persist", bufs=1))

        R_all = persist.tile([128, NT * BPC, RW], bf16, name="R_all")
        KT = persist.tile([64, 2 * L], bf16, name="KT")
        KbT = persist.tile([64, 2 * L], bf16, name="KbT")
        consts = persist.tile([128, 1536], bf16, name="consts")
        fpack = persist.tile([128, 131], f32, name="fpack")
        Mt_f32 = persist.tile([64, 2 * H], f32, name="Mt_f32")
        ctx_sb = persist.tile([64, 2], f32, name="ctx_sb")
        y_sb = persist.tile([1, 2 * V], f32, name="y_sb")

        I128 = consts[:, 0:128]
        I64 = consts[0:64, 0:64]
        M_BDL = consts[:, 128:640]     # [128,512] quad mask, -1 strict-lower blkdiag
        M_BDU = consts[:, 640:1152]    # -1 strict-upper blkdiag
        M_OFF = consts[:, 1152:1536]   # [128,384] quad mask, +1 off-block cols 0:96
        WrWo = fpack[0:64, 0:64]
        ONES11 = fpack[0:1, 66:67]
        BIAS2 = fpack[0:1, 67:131]

        # ---------------- DMAs (group-pipelined) ----------------
        nc.sync.dma_start(consts[:], consts_p[:])
        for g in range(NT // GP):
            c0 = g * GP
            tok = c0 * 128
            for b in range(2):
                o = b * L + tok
                nc.sync.dma_start(KT[:, o:o + GP * 128], kt_p[:, o:o + GP * 128])
                nc.sync.dma_start(KbT[:, o:o + GP * 128], kbt_p[:, o:o + GP * 128])
            nc.sync.dma_start(
                R_all[:, 2 * c0:2 * (c0 + GP), 0:128],
                kkbn_p[:, c0 * 256:(c0 + GP) * 256])
        nc.sync.dma_start(fpack[:], fpack_p[:])

        # ---------------- pools ----------------
        pp = est.enter_context(tc.tile_pool(name="pp", bufs=6, space="PSUM"))
        sb_sbd = est.enter_context(tc.tile_pool(name="sb_sbd", bufs=6))
        sb_abd = est.enter_context(tc.tile_pool(name="sb_abd", bufs=6))
        sb_s2 = est.enter_context(tc.tile_pool(name="sb_s2", bufs=6))
        sb_x1 = est.enter_context(tc.tile_pool(name="sb_x1", bufs=6))
        sb_x2 = est.enter_context(tc.tile_pool(name="sb_x2", bufs=6))
        sb_x3 = est.enter_context(tc.tile_pool(name="sb_x3", bufs=6))
        sb_nt = est.enter_context(tc.tile_pool(name="sb_nt", bufs=6))
        sb_v1 = est.enter_context(tc.tile_pool(name="sb_v1", bufs=4))
        sb_v2 = est.enter_context(tc.tile_pool(name="sb_v2", bufs=4))
        sb_v3 = est.enter_context(tc.tile_pool(name="sb_v3", bufs=6))
        sb_zk = est.enter_context(tc.tile_pool(name="sb_zk", bufs=4))
        sb_mt = est.enter_context(tc.tile_pool(name="sb_mt", bufs=4))

        st = [dict() for _ in range(NT)]   # per-pair tile state
        mt_prev = [None]                   # rolling Mt tile

        def ring(shape, name):
            return pp.tile(shape, f32, name=name, tag="ring", bufs=6)

        # ---------------- stages ----------------
        # quad stages (q = even pair index, covers pairs q and q+1;
        # slot j = 2*(cc - q) + b)
        def sA(q):
            for cc in (q, q + 1):
                ev = (cc % 2 == 0)
                st[cc]["rt"] = {"r1": ev, "r2": not ev, "r3": not ev,
                                "h1": ev, "h2": not ev, "h3": ev,
                                "zk": ev, "mt": not ev}
            ps_a = ring([128, 4, 128], "ps_a")
            ps_s = ring([128, 4, 128], "ps_s")
            for cc in (q, q + 1):
                for b in range(2):
                    j = 2 * (cc - q) + b
                    kts = KT[:, b * L + cc * 128:b * L + (cc + 1) * 128]
                    kbts = KbT[:, b * L + cc * 128:b * L + (cc + 1) * 128]
                    nc.tensor.matmul(ps_a[:, j, :], lhsT=kbts, rhs=kts,
                                     start=True, stop=True)
                    nc.tensor.matmul(ps_s[:, j, :], lhsT=kts, rhs=kbts,
                                     start=True, stop=True)
            st[q]["ps_a"] = ps_a
            st[q]["ps_s"] = ps_s

        def sMask(q):
            ps_a, ps_s = st[q]["ps_a"], st[q]["ps_s"]
            sbd = sb_sbd.tile([128, 4, 128], bf16, name="sbd")
            nc.vector.tensor_mul(sbd[:, :, :], ps_s[:, :, :], M_BDU)
            abd = sb_abd.tile([128, 4, 128], bf16, name="abd")
            nc.vector.tensor_mul(abd[:, :, :], ps_a[:, :, :], M_BDL)
            nc.vector.tensor_mul(
                R_all[:, 2 * q:2 * q + 4, 128:224], ps_a[:, :, 0:96], M_OFF)
            st[q]["sbd"] = sbd
            st[q]["abd"] = abd

        def sS2(q):
            sbd, abd = st[q]["sbd"], st[q]["abd"]
            ps = ring([128, 4, 128], "ps_s2")
            for j in range(4):
                nc.tensor.matmul(ps[:, j, :], lhsT=abd[:, j, :], rhs=sbd[:, j, :],
                                 start=True, stop=True)
            s2 = sb_s2.tile([128, 4, 128], bf16, name="s2")
            nc.scalar.copy(s2[:, :, :], ps[:, :, :])
            st[q]["s2"] = s2

        def _sl(c, key, b):
            """[128,128] slot slice of a quad tile for pair c, batch b."""
            q = c - (c % 2)
            t = st[q][key]
            return t[:, 2 * (c - q) + b, :]

        def _round(c, name, pool, lhs_of_b, add_in, base_in):
            """One solve round: out = base_in + L-ish @ add_in.
            Route D: bare mm + DVE tensor_add; route A: identity-fold + Act copy."""
            ps = ring([128, 2, RW], "ps_" + name)
            on_act = st[c]["rt"][name]
            for b in range(2):
                if on_act:
                    nc.tensor.matmul(ps[:, b, :], lhsT=I128, rhs=base_in(b),
                                     start=True, stop=False)
                nc.tensor.matmul(ps[:, b, :], lhsT=lhs_of_b(b), rhs=add_in(b),
                                 start=not on_act, stop=True)
            out = pool.tile([128, 2, RW], bf16, name=name)
            if on_act:
                nc.scalar.copy(out[:, :, :], ps[:, :, :])
            else:
                nc.vector.tensor_add(out[:, :, :], st[c]["base_ap"](), ps[:, :, :])
            return out

        def sR1(c):
            st[c]["base_ap"] = lambda: R_all[:, 2 * c:2 * c + 2, :]
            x1 = _round(c, "r1", sb_x1,
                        lambda b: _sl(c, "sbd", b),
                        lambda b: R_all[:, 2 * c + b, :],
                        lambda b: R_all[:, 2 * c + b, :])
            st[c]["x1"] = x1

        def sR2(c):
            x1 = st[c]["x1"]
            st[c]["base_ap"] = lambda: x1[:, :, :]
            x2 = _round(c, "r2", sb_x2,
                        lambda b: _sl(c, "s2", b),
                        lambda b: x1[:, b, :],
                        lambda b: x1[:, b, :])
            st[c]["x2"] = x2

        def sR3(c):
            x1, x2 = st[c]["x1"], st[c]["x2"]
            st[c]["base_ap"] = lambda: x1[:, :, :]
            x3 = _round(c, "r3", sb_x3,
                        lambda b: _sl(c, "s2", b),
                        lambda b: x2[:, b, :],
                        lambda b: x1[:, b, :])
            st[c]["x3"] = x3

        def sNT(q):
            ps = ring([96, 4, 128], "ps_nt")
            for cc in (q, q + 1):
                x3 = st[cc]["x3"]
                for b in range(2):
                    j = 2 * (cc - q) + b
                    nc.tensor.matmul(ps[:, j, :], lhsT=x3[:, b, 128:224], rhs=I128,
                                     start=True, stop=True)
            ntn = sb_nt.tile([96, 4, 128], bf16, name="ntn")
            nc.scalar.activation(ntn[:, :, :], ps[:, :, :], Act.Copy, scale=-1.0)
            st[q]["ntn"] = ntn

        def _horner(c, name, pool, vin_of_b):
            x3 = st[c]["x3"]
            ps = ring([128, 2, 128], "ps_" + name)
            on_act = st[c]["rt"][name]
            for b in range(2):
                if on_act:
                    nc.tensor.matmul(ps[:, b, :], lhsT=I128, rhs=x3[:, b, 0:128],
                                     start=True, stop=False)
                nc.tensor.matmul(ps[:, b, :], lhsT=_sl(c, "ntn", b),
                                 rhs=vin_of_b(b), start=not on_act, stop=True)
            out = pool.tile([128, 2, 128], bf16, name=name)
            if on_act:
                nc.scalar.copy(out[:, :, :], ps[:, :, :])
            else:
                nc.vector.tensor_add(out[:, :, :], x3[:, :, 0:128], ps[:, :, :])
            return out

        def sH1(c):
            x3 = st[c]["x3"]
            st[c]["v1"] = _horner(c, "h1", sb_v1, lambda b: x3[0:96, b, 0:128])

        def sH2(c):
            v1 = st[c]["v1"]
            st[c]["v2"] = _horner(c, "h2", sb_v2, lambda b: v1[0:96, b, :])

        def sH3(c):
            v2 = st[c]["v2"]
            st[c]["v3"] = _horner(c, "h3", sb_v3, lambda b: v2[0:96, b, :])

        def sZK(q):
            ps = ring([64, 4, 64], "ps_zk")
            for cc in (q, q + 1):
                v3 = st[cc]["v3"]
                for b in range(2):
                    j = 2 * (cc - q) + b
                    nc.tensor.matmul(ps[:, j, :], lhsT=v3[:, b, 64:128],
                                     rhs=R_all[:, 2 * cc + b, 0:64],
                                     start=True, stop=True)
            zkn = sb_zk.tile([64, 4, 64], bf16, name="zkn")
            if st[q]["rt"]["zk"]:
                nc.scalar.copy(zkn[:, :, :], ps[:, :, :])
            else:
                nc.vector.tensor_copy(zkn[:, :, :], ps[:, :, :])
            st[q]["zkn"] = zkn

        def s12(c):
            v3 = st[c]["v3"]
            ps = pp.tile([64, 2, 64], f32, name="ps_upd", tag="upd", bufs=2)
            for b in range(2):
                nc.tensor.matmul(ps[:, b, :], lhsT=R_all[:, 2 * c + b, 0:64],
                                 rhs=v3[:, b, 0:64], start=True, stop=(c == 0))
                if c > 0:
                    q = c - (c % 2)
                    zkn = st[q]["zkn"]
                    nc.tensor.matmul(ps[:, b, :],
                                     lhsT=zkn[0:64, 2 * (c - q) + b, :],
                                     rhs=mt_prev[0][:, b, :], start=False, stop=False)
                    nc.tensor.matmul(ps[:, b, :], lhsT=I64,
                                     rhs=mt_prev[0][:, b, :], start=False, stop=True)
            if c == NT - 1:
                nc.vector.tensor_copy(Mt_f32[:, :], ps[:, :, :])
            else:
                mt = sb_mt.tile([64, 2, 64], bf16, name="mt")
                if st[c]["rt"]["mt"]:
                    nc.scalar.copy(mt[:, :, :], ps[:, :, :])
                else:
                    nc.vector.tensor_copy(mt[:, :, :], ps[:, :, :])
                mt_prev[0] = mt

        # Software-pipelined emission: group g+1's head (masks/rounds,
        # DVE-lean) overlaps group g's tail (Horner copies, Act-lean).
        def _slot(stage, quad):
            def run(g):
                step = 2 if quad else 1
                for c in range(GP * g, GP * (g + 1), step):
                    stage(c)
            return run

        slots = [_slot(sA, True), _slot(sMask, True), _slot(sS2, True),
                 _slot(sR1, False), _slot(sR2, False), _slot(sR3, False),
                 _slot(sNT, True), _slot(sH1, False), _slot(sH2, False),
                 _slot(sH3, False), _slot(sZK, True)]
        for k in range(GP):
            slots.append(lambda g, k=k: s12(GP * g + k))

        NG = NT // GP
        import os as _os
        OFF = int(_os.environ.get("KN_OFF", "5"))
        for t in range(len(slots) + OFF * (NG - 1)):
            for g in range(NG):
                si = t - OFF * g
                if 0 <= si < len(slots):
                    slots[si](g)

        # ---------------- readout ----------------
        ps_ctx = pp.tile([64, 2], f32, name="ps_ctx", tag="ring", bufs=6)
        for b in range(2):
            nc.tensor.matmul(ps_ctx[:, b:b + 1], lhsT=Mt_f32[:, 64 * b:64 * (b + 1)],
                             rhs=fpack[0:64, 64 + b:65 + b], start=True, stop=True)
        nc.vector.tensor_copy(ctx_sb[:], ps_ctx[:])
        ps_y = pp.tile([1, 2 * V], f32, name="ps_y", tag="ring", bufs=6)
        for b in range(2):
            nc.tensor.matmul(ps_y[0:1, 64 * b:64 * (b + 1)], lhsT=ctx_sb[:, b:b + 1],
                             rhs=WrWo, start=True, stop=False)
            nc.tensor.matmul(ps_y[0:1, 64 * b:64 * (b + 1)], lhsT=ONES11,
                             rhs=BIAS2, start=False, stop=True)
        nc.vector.tensor_copy(y_sb[:], ps_y[:])
        for b in range(2):
            nc.sync.dma_start(out_p[b:b + 1, :], y_sb[0:1, 64 * b:64 * (b + 1)])

    if legalize:
        _legalize_waits(nc, mybir)
    return nc


def _legalize_waits(nc, mybir):
    """This walrus build encodes at most one sync-wait per instruction.
    Split multi-wait instructions into single-wait NoOp prefixes on the
    same engine (engine queues execute in order, so semantics hold)."""
    k = 0
    for blk in nc.main_func.blocks:
        insts = blk.instructions
        out = []
        changed = False
        for inst in list(insts):
            si = inst.sync_info
            waits = list(si.on_wait) if si is not None and si.on_wait else []
            if len(waits) > 1:
                for w in waits[:-1]:
                    nop = mybir.InstNoOp(name=f"I-wsplit-{k}", ins=[], outs=[])
                    k += 1
                    nop.engine = inst.engine
                    nop.sync_info = mybir.SyncInfo(on_wait=[w], on_update=[])
                    out.append(nop)
                si.on_wait = [waits[-1]]
                changed = True
            out.append(inst)
        if changed:
            while len(insts):
                insts.pop()
            for x in out:
                insts.append(x)


def host_prep(inputs):
    """Fold the vocab table and gather per-token rows; returns per-core maps."""
    import ml_dtypes
    bf = ml_dtypes.bfloat16
    f64 = np.float64
    g = lambda k: np.asarray(inputs[k], f64)
    embed, W1, b1, W2, b2 = g("embed"), g("W1"), g("b1"), g("W2"), g("b2")
    gamma, beta, Wr, br, Wo, bo = (g("gamma"), g("beta"), g("Wr"), g("br"),
                                   g("Wo"), g("bo"))
    seq = np.asarray(inputs["seq"], np.int64)

    ff = np.maximum(embed @ W1 + b1, 0) @ W2 + b2
    x = embed + ff
    mu = x.mean(-1, keepdims=True)
    var = x.var(-1, keepdims=True)
    xln = ((x - mu) / np.sqrt(var + LN_EPS) * gamma + beta).astype(np.float32)
    betav = (1.0 / ((xln.astype(f64) ** 2).sum(-1) + D_EPS)).astype(np.float32)
    xlnb = xln * betav[:, None]
    WrWo = (Wr @ Wo).astype(np.float32)
    bias2 = (br @ Wo + bo).astype(np.float32)

    # constants pack
    t = np.arange(C)
    blk = t // NBLK
    m_bdl = -((t[:, None] > t[None, :]) & (blk[:, None] == blk[None, :])).astype(np.float32)
    m_bdu = m_bdl.T.copy()
    m_off = ((t[:, None] > t[None, :]) & (blk[:, None] != blk[None, :])).astype(np.float32)[:, :96]
    consts = np.zeros((128, 1536), np.float32)
    consts[:, 0:128] = np.eye(128)
    for j in range(4):
        consts[:, 128 + 128 * j:256 + 128 * j] = m_bdl
        consts[:, 640 + 128 * j:768 + 128 * j] = m_bdu
        consts[:, 1152 + 96 * j:1248 + 96 * j] = m_off
    consts_bf = consts.astype(bf)

    in_maps = []
    for core in range(NCORES):
        sq = seq[core * BPC:(core + 1) * BPC]        # [2, 2048]
        K = xln[sq].astype(bf).astype(np.float32)    # [2, L, 64] bf16-rounded
        Kb = xlnb[sq].astype(bf).astype(np.float32)
        q = xln[sq[:, L - 1]].astype(np.float32)     # [2, 64] before zeroing
        K[:, L - 1, :] = 0.0
        Kb[:, L - 1, :] = 0.0

        kkbn = np.empty((2, NT, C, 128), np.float32)
        kkbn[:, :, :, 0:64] = K.reshape(2, NT, C, 64)
        kkbn[:, :, :, 64:128] = -Kb.reshape(2, NT, C, 64)
        # device layout [128(t), c, b, col]
        kkbn = np.ascontiguousarray(kkbn.transpose(2, 1, 0, 3)).reshape(128, NT * BPC * 128)

        kt = np.concatenate([K[0].T, K[1].T], axis=1)     # [64, 2L]
        kbt = np.concatenate([Kb[0].T, Kb[1].T], axis=1)

        fpack = np.zeros((128, 131), np.float32)
        fpack[0:64, 0:64] = WrWo
        fpack[0:64, 64] = q[0]
        fpack[0:64, 65] = q[1]
        fpack[0, 66] = 1.0
        fpack[0, 67:131] = bias2
        in_maps.append({
            "kkbn": kkbn.astype(bf),
            "kt": kt.astype(bf),
            "kbt": kbt.astype(bf),
            "consts": consts_bf,
            "fpack": fpack,
        })
    return in_maps


def _get_nc():
    if "nc" not in _CACHE:
        _CACHE["nc"] = _build_nc()
    return _CACHE["nc"]


def kernel(**inputs):
    from concourse.bass_utils import run_bass_kernel_spmd

    nc = _get_nc()
    in_maps = host_prep(inputs)
    res = run_bass_kernel_spmd(nc, in_maps, core_ids=list(range(NCORES)))
    out = np.concatenate([r["out"] for r in res.results], axis=0)
    return out.astype(np.float32)


if __name__ == "__main__":
    d = np.load("/root/problem/inputs.npz")
    y = kernel(**{k: d[k] for k in d.files})
    o = np.load("/root/problem/oracle.npz")
    rel = np.abs(y - o["y"]).max() / np.abs(o["y"]).max()
    print("Relative error:", rel)


# revision 4
# speedup vs baseline: 7.1913x; 1.0077x over previous
"""DeltaModel Trainium2 kernel, v2.

Host folds the vocab-64 token table (embed->FFN->LayerNorm), gathers per-token
K / beta*K rows, and ships them in both token-major and feature-major layouts.
Device runs, per 128-token chunk (2 batch elems/core paired side by side):
  A = Kb K^T, S = A^T (swapped-operand matmuls)
  block-diag strict-lower L = A_bd; solve (I+L)^-1 via exact degree-5
  Neumann product (I + L^2 + L^4)(I - L) -> X = [W | -Z | N-part]
  outer correction (I+N)^-1 = I - N + N^2 - N^3 (exact, N^4 = 0)
  Mt <- Mt + K^T W - (K^T Z) Mt   (PSUM-accumulated, chained over chunks)
  readout y = (Mt^T q) (Wr Wo) + (br Wo + bo) in fp32.
Emission is stage-major across chunk pairs (4 groups of 4) so independent
chunks hide cross-engine semaphore latency; PSUM->SBUF traffic is balanced
across DVE / Act / Pool.
"""

import numpy as np

H = 64
V = 64
B = 16
L = 2048
NCORES = 8
BPC = 2                 # batch per core
NT = 16                 # chunks per batch elem
C = 128                 # chunk length
NBLK = 32               # jacobi block
RW = 224                # R width: K(64) | -Kb(64) | A_off(96)
GP = 4                  # pairs per group
LN_EPS = 1e-5
D_EPS = 1e-6

_CACHE = {}


def _build_nc(legalize=True):
    import concourse.bass as bass
    import concourse.mybir as mybir
    import concourse.tile as tile

    dt = mybir.dt
    f32 = dt.float32
    bf16 = dt.bfloat16
    Act = mybir.ActivationFunctionType

    nc = bass.Bass()

    kkbn_p = nc.declare_dram_parameter("kkbn", [128, NT * BPC * 128], bf16, isOutput=False)
    kt_p = nc.declare_dram_parameter("kt", [64, 2 * L], bf16, isOutput=False)
    kbt_p = nc.declare_dram_parameter("kbt", [64, 2 * L], bf16, isOutput=False)
    consts_p = nc.declare_dram_parameter("consts", [128, 1536], bf16, isOutput=False)
    fpack_p = nc.declare_dram_parameter("fpack", [128, 131], f32, isOutput=False)
    out_p = nc.declare_dram_parameter("out", [BPC, V], f32, isOutput=True)

    from contextlib import ExitStack
    with tile.TileContext(nc) as tc, ExitStack() as est:
        persist = est.enter_context(tc.tile_pool(name="persist", bufs=1))

        R_all = persist.tile([128, NT * BPC, RW], bf16, name="R_all")
        KT = persist.tile([64, 2 * L], bf16, name="KT")
        KbT = persist.tile([64, 2 * L], bf16, name="KbT")
        consts = persist.tile([128, 1536], bf16, name="consts")
        fpack = persist.tile([128, 131], f32, name="fpack")
        Mt_f32 = persist.tile([64, 2 * H], f32, name="Mt_f32")
        ctx_sb = persist.tile([64, 2], f32, name="ctx_sb")
        y_sb = persist.tile([1, 2 * V], f32, name="y_sb")

        I128 = consts[:, 0:128]
        I64 = consts[0:64, 0:64]
        M_BDL = consts[:, 128:640]     # [128,512] quad mask, -1 strict-lower blkdiag
        M_BDU = consts[:, 640:1152]    # -1 strict-upper blkdiag
        M_OFF = consts[:, 1152:1536]   # [128,384] quad mask, +1 off-block cols 0:96
        WrWo = fpack[0:64, 0:64]
        ONES11 = fpack[0:1, 66:67]
        BIAS2 = fpack[0:1, 67:131]

        # ---------------- DMAs (group-pipelined) ----------------
        # group-0 operands first so compute starts ASAP; consts (needed by
        # the first sMask) next; later groups stream under compute
        for g in range(NT // GP):
            c0 = g * GP
            tok = c0 * 128
            for b in range(2):
                o = b * L + tok
                nc.sync.dma_start(KT[:, o:o + GP * 128], kt_p[:, o:o + GP * 128])
                nc.sync.dma_start(KbT[:, o:o + GP * 128], kbt_p[:, o:o + GP * 128])
            if g == 0:
                nc.sync.dma_start(consts[:], consts_p[:])
            nc.sync.dma_start(
                R_all[:, 2 * c0:2 * (c0 + GP), 0:128],
                kkbn_p[:, c0 * 256:(c0 + GP) * 256])
        nc.sync.dma_start(fpack[:], fpack_p[:])

        # ---------------- pools ----------------
        pp = est.enter_context(tc.tile_pool(name="pp", bufs=6, space="PSUM"))
        sb_sbd = est.enter_context(tc.tile_pool(name="sb_sbd", bufs=6))
        sb_abd = est.enter_context(tc.tile_pool(name="sb_abd", bufs=6))
        sb_s2 = est.enter_context(tc.tile_pool(name="sb_s2", bufs=6))
        sb_x1 = est.enter_context(tc.tile_pool(name="sb_x1", bufs=6))
        sb_x2 = est.enter_context(tc.tile_pool(name="sb_x2", bufs=6))
        sb_x3 = est.enter_context(tc.tile_pool(name="sb_x3", bufs=6))
        sb_nt = est.enter_context(tc.tile_pool(name="sb_nt", bufs=6))
        sb_v1 = est.enter_context(tc.tile_pool(name="sb_v1", bufs=4))
        sb_v2 = est.enter_context(tc.tile_pool(name="sb_v2", bufs=4))
        sb_v3 = est.enter_context(tc.tile_pool(name="sb_v3", bufs=6))
        sb_zk = est.enter_context(tc.tile_pool(name="sb_zk", bufs=4))
        sb_mt = est.enter_context(tc.tile_pool(name="sb_mt", bufs=4))

        st = [dict() for _ in range(NT)]   # per-pair tile state
        mt_prev = [None]                   # rolling Mt tile

        def ring(shape, name):
            return pp.tile(shape, f32, name=name, tag="ring", bufs=6)

        # ---------------- stages ----------------
        # quad stages (q = even pair index, covers pairs q and q+1;
        # slot j = 2*(cc - q) + b)
        def sA(q):
            for cc in (q, q + 1):
                ev = (cc % 2 == 0)
                st[cc]["rt"] = {"r1": ev, "r2": not ev, "r3": not ev,
                                "h1": ev, "h2": not ev, "h3": ev,
                                "zk": ev, "mt": not ev}
            ps_a = ring([128, 4, 128], "ps_a")
            ps_s = ring([128, 4, 128], "ps_s")
            for cc in (q, q + 1):
                for b in range(2):
                    j = 2 * (cc - q) + b
                    kts = KT[:, b * L + cc * 128:b * L + (cc + 1) * 128]
                    kbts = KbT[:, b * L + cc * 128:b * L + (cc + 1) * 128]
                    nc.tensor.matmul(ps_a[:, j, :], lhsT=kbts, rhs=kts,
                                     start=True, stop=True)
                    nc.tensor.matmul(ps_s[:, j, :], lhsT=kts, rhs=kbts,
                                     start=True, stop=True)
            st[q]["ps_a"] = ps_a
            st[q]["ps_s"] = ps_s

        def sMask(q):
            ps_a, ps_s = st[q]["ps_a"], st[q]["ps_s"]
            sbd = sb_sbd.tile([128, 4, 128], bf16, name="sbd")
            nc.vector.tensor_mul(sbd[:, :, :], ps_s[:, :, :], M_BDU)
            abd = sb_abd.tile([128, 4, 128], bf16, name="abd")
            nc.vector.tensor_mul(abd[:, :, :], ps_a[:, :, :], M_BDL)
            nc.vector.tensor_mul(
                R_all[:, 2 * q:2 * q + 4, 128:224], ps_a[:, :, 0:96], M_OFF)
            st[q]["sbd"] = sbd
            st[q]["abd"] = abd

        def sS2(q):
            sbd, abd = st[q]["sbd"], st[q]["abd"]
            ps = ring([128, 4, 128], "ps_s2")
            for j in range(4):
                nc.tensor.matmul(ps[:, j, :], lhsT=abd[:, j, :], rhs=sbd[:, j, :],
                                 start=True, stop=True)
            s2 = sb_s2.tile([128, 4, 128], bf16, name="s2")
            nc.scalar.copy(s2[:, :, :], ps[:, :, :])
            st[q]["s2"] = s2

        def _sl(c, key, b):
            """[128,128] slot slice of a quad tile for pair c, batch b."""
            q = c - (c % 2)
            t = st[q][key]
            return t[:, 2 * (c - q) + b, :]

        def _round(c, name, pool, lhs_of_b, add_in, base_in):
            """One solve round: out = base_in + L-ish @ add_in.
            Route D: bare mm + DVE tensor_add; route A: identity-fold + Act copy."""
            ps = ring([128, 2, RW], "ps_" + name)
            on_act = st[c]["rt"][name]
            for b in range(2):
                if on_act:
                    nc.tensor.matmul(ps[:, b, :], lhsT=I128, rhs=base_in(b),
                                     start=True, stop=False)
                nc.tensor.matmul(ps[:, b, :], lhsT=lhs_of_b(b), rhs=add_in(b),
                                 start=not on_act, stop=True)
            out = pool.tile([128, 2, RW], bf16, name=name)
            if on_act:
                nc.scalar.copy(out[:, :, :], ps[:, :, :])
            else:
                nc.vector.tensor_add(out[:, :, :], st[c]["base_ap"](), ps[:, :, :])
            return out

        def sR1(c):
            st[c]["base_ap"] = lambda: R_all[:, 2 * c:2 * c + 2, :]
            x1 = _round(c, "r1", sb_x1,
                        lambda b: _sl(c, "sbd", b),
                        lambda b: R_all[:, 2 * c + b, :],
                        lambda b: R_all[:, 2 * c + b, :])
            st[c]["x1"] = x1

        def sR2(c):
            x1 = st[c]["x1"]
            st[c]["base_ap"] = lambda: x1[:, :, :]
            x2 = _round(c, "r2", sb_x2,
                        lambda b: _sl(c, "s2", b),
                        lambda b: x1[:, b, :],
                        lambda b: x1[:, b, :])
            st[c]["x2"] = x2

        def sR3(c):
            x1, x2 = st[c]["x1"], st[c]["x2"]
            st[c]["base_ap"] = lambda: x1[:, :, :]
            x3 = _round(c, "r3", sb_x3,
                        lambda b: _sl(c, "s2", b),
                        lambda b: x2[:, b, :],
                        lambda b: x1[:, b, :])
            st[c]["x3"] = x3

        def sNT(q):
            ps = ring([96, 4, 128], "ps_nt")
            for cc in (q, q + 1):
                x3 = st[cc]["x3"]
                for b in range(2):
                    j = 2 * (cc - q) + b
                    nc.tensor.matmul(ps[:, j, :], lhsT=x3[:, b, 128:224], rhs=I128,
                                     start=True, stop=True)
            ntn = sb_nt.tile([96, 4, 128], bf16, name="ntn")
            nc.scalar.activation(ntn[:, :, :], ps[:, :, :], Act.Copy, scale=-1.0)
            st[q]["ntn"] = ntn

        def _horner(c, name, pool, vin_of_b):
            x3 = st[c]["x3"]
            ps = ring([128, 2, 128], "ps_" + name)
            on_act = st[c]["rt"][name]
            for b in range(2):
                if on_act:
                    nc.tensor.matmul(ps[:, b, :], lhsT=I128, rhs=x3[:, b, 0:128],
                                     start=True, stop=False)
                nc.tensor.matmul(ps[:, b, :], lhsT=_sl(c, "ntn", b),
                                 rhs=vin_of_b(b), start=not on_act, stop=True)
            out = pool.tile([128, 2, 128], bf16, name=name)
            if on_act:
                nc.scalar.copy(out[:, :, :], ps[:, :, :])
            else:
                nc.vector.tensor_add(out[:, :, :], x3[:, :, 0:128], ps[:, :, :])
            return out

        def sH1(c):
            x3 = st[c]["x3"]
            st[c]["v1"] = _horner(c, "h1", sb_v1, lambda b: x3[0:96, b, 0:128])

        def sH2(c):
            v1 = st[c]["v1"]
            st[c]["v2"] = _horner(c, "h2", sb_v2, lambda b: v1[0:96, b, :])

        def sH3(c):
            v2 = st[c]["v2"]
            st[c]["v3"] = _horner(c, "h3", sb_v3, lambda b: v2[0:96, b, :])

        def sZK(q):
            ps = ring([64, 4, 64], "ps_zk")
            for cc in (q, q + 1):
                v3 = st[cc]["v3"]
                for b in range(2):
                    j = 2 * (cc - q) + b
                    nc.tensor.matmul(ps[:, j, :], lhsT=v3[:, b, 64:128],
                                     rhs=R_all[:, 2 * cc + b, 0:64],
                                     start=True, stop=True)
            zkn = sb_zk.tile([64, 4, 64], bf16, name="zkn")
            if st[q]["rt"]["zk"]:
                nc.scalar.copy(zkn[:, :, :], ps[:, :, :])
            else:
                nc.vector.tensor_copy(zkn[:, :, :], ps[:, :, :])
            st[q]["zkn"] = zkn

        def s12(c):
            v3 = st[c]["v3"]
            ps = pp.tile([64, 2, 64], f32, name="ps_upd", tag="upd", bufs=2)
            for b in range(2):
                nc.tensor.matmul(ps[:, b, :], lhsT=R_all[:, 2 * c + b, 0:64],
                                 rhs=v3[:, b, 0:64], start=True, stop=(c == 0))
                if c > 0:
                    q = c - (c % 2)
                    zkn = st[q]["zkn"]
                    nc.tensor.matmul(ps[:, b, :],
                                     lhsT=zkn[0:64, 2 * (c - q) + b, :],
                                     rhs=mt_prev[0][:, b, :], start=False, stop=False)
                    nc.tensor.matmul(ps[:, b, :], lhsT=I64,
                                     rhs=mt_prev[0][:, b, :], start=False, stop=True)
            if c == NT - 1:
                nc.vector.tensor_copy(Mt_f32[:, :], ps[:, :, :])
            else:
                mt = sb_mt.tile([64, 2, 64], bf16, name="mt")
                if st[c]["rt"]["mt"]:
                    nc.scalar.copy(mt[:, :, :], ps[:, :, :])
                else:
                    nc.vector.tensor_copy(mt[:, :, :], ps[:, :, :])
                mt_prev[0] = mt

        # Software-pipelined emission: group g+1's head (masks/rounds,
        # DVE-lean) overlaps group g's tail (Horner copies, Act-lean).
        def _slot(stage, quad):
            def run(g):
                step = 2 if quad else 1
                for c in range(GP * g, GP * (g + 1), step):
                    stage(c)
            return run

        slots = [_slot(sA, True), _slot(sMask, True), _slot(sS2, True),
                 _slot(sR1, False), _slot(sR2, False), _slot(sR3, False),
                 _slot(sNT, True), _slot(sH1, False), _slot(sH2, False),
                 _slot(sH3, False), _slot(sZK, True)]
        for k in range(GP):
            slots.append(lambda g, k=k: s12(GP * g + k))

        NG = NT // GP
        import os as _os
        OFF = int(_os.environ.get("KN_OFF", "5"))
        for t in range(len(slots) + OFF * (NG - 1)):
            for g in range(NG):
                si = t - OFF * g
                if 0 <= si < len(slots):
                    slots[si](g)

        # ---------------- readout ----------------
        ps_ctx = pp.tile([64, 2], f32, name="ps_ctx", tag="ring", bufs=6)
        for b in range(2):
            nc.tensor.matmul(ps_ctx[:, b:b + 1], lhsT=Mt_f32[:, 64 * b:64 * (b + 1)],
                             rhs=fpack[0:64, 64 + b:65 + b], start=True, stop=True)
        nc.vector.tensor_copy(ctx_sb[:], ps_ctx[:])
        ps_y = pp.tile([1, 2 * V], f32, name="ps_y", tag="ring", bufs=6)
        for b in range(2):
            nc.tensor.matmul(ps_y[0:1, 64 * b:64 * (b + 1)], lhsT=ctx_sb[:, b:b + 1],
                             rhs=WrWo, start=True, stop=False)
            nc.tensor.matmul(ps_y[0:1, 64 * b:64 * (b + 1)], lhsT=ONES11,
                             rhs=BIAS2, start=False, stop=True)
        nc.vector.tensor_copy(y_sb[:], ps_y[:])
        for b in range(2):
            nc.sync.dma_start(out_p[b:b + 1, :], y_sb[0:1, 64 * b:64 * (b + 1)])

    if legalize:
        _legalize_waits(nc, mybir)
    return nc


def _legalize_waits(nc, mybir):
    """This walrus build encodes at most one sync-wait per instruction.
    Split multi-wait instructions into single-wait NoOp prefixes on the
    same engine (engine queues execute in order, so semantics hold)."""
    k = 0
    for blk in nc.main_func.blocks:
        insts = blk.instructions
        out = []
        changed = False
        for inst in list(insts):
            si = inst.sync_info
            waits = list(si.on_wait) if si is not None and si.on_wait else []
            if len(waits) > 1:
                for w in waits[:-1]:
                    nop = mybir.InstNoOp(name=f"I-wsplit-{k}", ins=[], outs=[])
                    k += 1
                    nop.engine = inst.engine
                    nop.sync_info = mybir.SyncInfo(on_wait=[w], on_update=[])
                    out.append(nop)
                si.on_wait = [waits[-1]]
                changed = True
            out.append(inst)
        if changed:
            while len(insts):
                insts.pop()
            for x in out:
                insts.append(x)


def host_prep(inputs):
    """Fold the vocab table and gather per-token rows; returns per-core maps."""
    import ml_dtypes
    bf = ml_dtypes.bfloat16
    f64 = np.float64
    g = lambda k: np.asarray(inputs[k], f64)
    embed, W1, b1, W2, b2 = g("embed"), g("W1"), g("b1"), g("W2"), g("b2")
    gamma, beta, Wr, br, Wo, bo = (g("gamma"), g("beta"), g("Wr"), g("br"),
                                   g("Wo"), g("bo"))
    seq = np.asarray(inputs["seq"], np.int64)

    ff = np.maximum(embed @ W1 + b1, 0) @ W2 + b2
    x = embed + ff
    mu = x.mean(-1, keepdims=True)
    var = x.var(-1, keepdims=True)
    xln = ((x - mu) / np.sqrt(var + LN_EPS) * gamma + beta).astype(np.float32)
    betav = (1.0 / ((xln.astype(f64) ** 2).sum(-1) + D_EPS)).astype(np.float32)
    xlnb = xln * betav[:, None]
    WrWo = (Wr @ Wo).astype(np.float32)
    bias2 = (br @ Wo + bo).astype(np.float32)

    # constants pack
    t = np.arange(C)
    blk = t // NBLK
    m_bdl = -((t[:, None] > t[None, :]) & (blk[:, None] == blk[None, :])).astype(np.float32)
    m_bdu = m_bdl.T.copy()
    m_off = ((t[:, None] > t[None, :]) & (blk[:, None] != blk[None, :])).astype(np.float32)[:, :96]
    consts = np.zeros((128, 1536), np.float32)
    consts[:, 0:128] = np.eye(128)
    for j in range(4):
        consts[:, 128 + 128 * j:256 + 128 * j] = m_bdl
        consts[:, 640 + 128 * j:768 + 128 * j] = m_bdu
        consts[:, 1152 + 96 * j:1248 + 96 * j] = m_off
    consts_bf = consts.astype(bf)

    in_maps = []
    for core in range(NCORES):
        sq = seq[core * BPC:(core + 1) * BPC]        # [2, 2048]
        K = xln[sq].astype(bf).astype(np.float32)    # [2, L, 64] bf16-rounded
        Kb = xlnb[sq].astype(bf).astype(np.float32)
        q = xln[sq[:, L - 1]].astype(np.float32)     # [2, 64] before zeroing
        K[:, L - 1, :] = 0.0
        Kb[:, L - 1, :] = 0.0

        kkbn = np.empty((2, NT, C, 128), np.float32)
        kkbn[:, :, :, 0:64] = K.reshape(2, NT, C, 64)
        kkbn[:, :, :, 64:128] = -Kb.reshape(2, NT, C, 64)
        # device layout [128(t), c, b, col]
        kkbn = np.ascontiguousarray(kkbn.transpose(2, 1, 0, 3)).reshape(128, NT * BPC * 128)

        kt = np.concatenate([K[0].T, K[1].T], axis=1)     # [64, 2L]
        kbt = np.concatenate([Kb[0].T, Kb[1].T], axis=1)

        fpack = np.zeros((128, 131), np.float32)
        fpack[0:64, 0:64] = WrWo
        fpack[0:64, 64] = q[0]
        fpack[0:64, 65] = q[1]
        fpack[0, 66] = 1.0
        fpack[0, 67:131] = bias2
        in_maps.append({
            "kkbn": kkbn.astype(bf),
            "kt": kt.astype(bf),
            "kbt": kbt.astype(bf),
            "consts": consts_bf,
            "fpack": fpack,
        })
    return in_maps


def _get_nc():
    if "nc" not in _CACHE:
        _CACHE["nc"] = _build_nc()
    return _CACHE["nc"]


def kernel(**inputs):
    from concourse.bass_utils import run_bass_kernel_spmd

    nc = _get_nc()
    in_maps = host_prep(inputs)
    res = run_bass_kernel_spmd(nc, in_maps, core_ids=list(range(NCORES)))
    out = np.concatenate([r["out"] for r in res.results], axis=0)
    return out.astype(np.float32)


if __name__ == "__main__":
    d = np.load("/root/problem/inputs.npz")
    y = kernel(**{k: d[k] for k in d.files})
    o = np.load("/root/problem/oracle.npz")
    rel = np.abs(y - o["y"]).max() / np.abs(o["y"]).max()
    print("Relative error:", rel)
